# revision 1
# baseline (speedup 1.0000x reference)
"""Trainium2 Bass kernel for GCN message passing (COO SpMM segment-sum).

out[i] = sum_{e: rows[e]==i} vals[e] * embeds[cols[e]]
N=100000 nodes, E=1600000 edges, D=64 features, f32 in/out.

Strategy (8 NeuronCores, SPMD, no collectives):
  - Shard OUTPUT rows across cores: core k owns rows [12500k, 12500(k+1))
    (rows are sorted, so each core's edges are one contiguous slice),
    split into W=98 windows of 128 output rows, processed in NS=14 spans
    of GW=7 windows.
  - embeds stored as bf16 PAIR tables: chunk h in {0,1} holds rows
    [50000h, 50000(h+1)) as entries of 2 consecutive rows = 128 bf16 =
    256B (dma_gather needs elem_size_bytes % 256 == 0). Edges grouped per
    (chunk h, col parity): group g = 2h + parity; gather index = pair
    code (col % 50000) // 2 < 25000 (int16-safe); the matmul rhs view
    takes the parity half of each gathered 128-wide slot.
  - Within each (group, span) gather call, the 7 windows' edge runs are
    packed CONTIGUOUSLY, each padded only to u[g,w] = max edge count
    over the 8 cores (NOT to a multiple of 128). 128-slot tile columns
    may span window boundaries; a boundary tile gets one matmul per
    window it touches, with the host-built one-hot M masked to that
    window's slots. This cuts gather descriptors ~14% vs per-window
    ceil-to-128 padding, and descriptors are the wall (see below).
  - Gathers are spread across 4 SWDGE queues (queue_num = g): each queue
    has its own Q7 descriptor-gen core-pair AND its own DMA ring, giving
    4 outstanding HBM reads per SDMA engine (4x latency hiding of the
    ~127ns random-read latency). Padding slots point at their run's LAST
    VALID index so pad fetches hit an already-open HBM row.
  - The one-hot scatter matrices M ([128 slots x 128 rloc] bf16 per
    matmul, M[p, rloc[p]] = val[p], zero outside the target window) are
    PRECOMPUTED ON HOST and streamed per (group, span) over HWDGE
    (nc.sync.dma_start). No DVE ops anywhere: DVE 2-port perf-mode ops
    lock GpSimd out of the shared SBUF port pair, starving SWDGE
    descriptor generation and serializing gather against compute.
  - psum[w] accumulates bf16 matmuls in f32 PSUM across the window's
    tile columns (start/stop flags from the host schedule), then one ACT
    copy psum->out_sb per window and a per-span output DMA.

Host prep (prep_shards) runs in numpy and is not part of device time.
"""

import os

import numpy as np
import ml_dtypes

BF16 = ml_dtypes.bfloat16

N_NODES = 100000
N_EDGES = 1600000
D = 64
P = 128
NC = 8
RPC = N_NODES // NC
W = -(-RPC // P)  # 98
NG = 4
HROWS = N_NODES // 2
NPAIR = HROWS // 2
GW = 7
NS = W // GW  # 14 spans

LAST_RESULTS = None


def _schedule(u):
    """Build call structure from u [NG, W] (slots per window per group).

    Returns dict with per-(g,s) offsets, tile counts, matmul schedules and
    global slot/M-column bases. Schedule entry: (w_in_span, ordinal)."""
    assert u.min() > 0
    off_gw = np.zeros((NG, W), np.int64)  # window offset within its call
    S_call = np.zeros((NG, NS), np.int64)
    T_call = np.zeros((NG, NS), np.int64)
    sched = {}
    n_mm = np.zeros((NG, NS), np.int64)
    for g in range(NG):
        for s in range(NS):
            ws = list(range(GW * s, GW * s + GW))
            off = 0
            for w in ws:
                off_gw[g, w] = off
                off += int(u[g, w])
            S_call[g, s] = off
            T = -(-off // P)
            T_call[g, s] = T
            ordinal = 0
            percall = []
            for C in range(T):
                lo, hi = C * P, C * P + P
                entries = []
                for i, w in enumerate(ws):
                    wlo = int(off_gw[g, w])
                    whi = wlo + int(u[g, w])
                    if wlo < hi and whi > lo:
                        entries.append((i, ordinal))
                        ordinal += 1
                percall.append(entries)
            sched[(g, s)] = percall
            n_mm[g, s] = ordinal
    # slot base per (g, s): group-major then span
    slot_base = np.zeros((NG, NS), np.int64)
    acc = 0
    for g in range(NG):
        for s in range(NS):
            slot_base[g, s] = acc
            acc += int(T_call[g, s]) * P
    n_slots = acc
    # M column base per (g, s): span-major then group (per-span locality)
    m_base = np.zeros((NG, NS), np.int64)
    acc = 0
    for s in range(NS):
        for g in range(NG):
            m_base[g, s] = acc
            acc += int(n_mm[g, s]) * P
    n_mcols = acc
    return dict(
        off_gw=off_gw,
        S_call=S_call,
        T_call=T_call,
        sched=sched,
        n_mm=n_mm,
        slot_base=slot_base,
        n_slots=n_slots,
        m_base=m_base,
        n_mcols=n_mcols,
    )


def build_program(u, reps=1, bufs_g=3, bufs_m=1, bufs_ps=8):
    import concourse.bacc as bacc
    import concourse.mybir as mybir
    import concourse.tile as tile

    f32 = mybir.dt.float32
    bf16 = mybir.dt.bfloat16
    i16 = mybir.dt.int16

    u = np.asarray(u)
    sc = _schedule(u)
    off_gw, T_call, sched = sc["off_gw"], sc["T_call"], sc["sched"]
    n_mm, slot_base, m_base = sc["n_mm"], sc["slot_base"], sc["m_base"]
    n_slots, n_mcols = sc["n_slots"], sc["n_mcols"]
    d = D

    # first/last matmul (g, C) of each window for start/stop flags
    first_gc = {}
    last_gc = {}
    for s in range(NS):
        for g in range(NG):
            for C, entries in enumerate(sched[(g, s)]):
                for i, _o in entries:
                    w = GW * s + i
                    if w not in first_gc:
                        first_gc[w] = (g, C)
                    last_gc[w] = (g, C)

    nc = bacc.Bacc(num_swdge_queues=4)
    tab_ds = [
        nc.declare_dram_parameter(f"tab{h}", [NPAIR, 2 * d], bf16, isOutput=False)
        for h in range(2)
    ]
    # pad idx cols to a multiple of 16 slots (always is: T*128)
    idx_d = nc.declare_dram_parameter("idx", [P, n_slots // 16], i16, isOutput=False)
    m_d = nc.declare_dram_parameter("m", [P, n_mcols], bf16, isOutput=False)
    out_d = nc.declare_dram_parameter("out", [P, W * d], f32, isOutput=True)

    with tile.TileContext(nc) as tc:
        with (
            tc.tile_pool(name="const", bufs=1) as cpool,
            tc.tile_pool(name="gath", bufs=bufs_g) as gpool,
            tc.tile_pool(name="mst", bufs=bufs_m) as mpool,
            tc.tile_pool(name="ps", bufs=bufs_ps, space="PSUM") as ppool,
        ):
            idx_sb = cpool.tile([P, n_slots // 16], i16, name="idx_sb")
            nc.sync.dma_start(out=idx_sb[:], in_=idx_d[:])
            out_sb = cpool.tile([P, W * d], f32, name="out_sb")

            def body():
                for s in range(NS):
                    gb3 = []
                    for g in range(NG):
                        h = g // 2
                        T = int(T_call[g, s])
                        gb = gpool.tile(
                            [P, T * 2 * d], bf16, name=f"gb{g}", tag=f"gb{g}"
                        )
                        view = gb[:].rearrange("p (n x) -> p n x", x=2 * d)
                        gb3.append(view)
                        n_idx = T * P
                        sb = int(slot_base[g, s])
                        nc.gpsimd.dma_gather(
                            out_ap=view,
                            in_ap=tab_ds[h][:, :],
                            idxs_ap=idx_sb[:, sb // 16 : (sb + n_idx) // 16],
                            num_idxs=n_idx,
                            num_idxs_reg=n_idx,
                            elem_size=2 * d,
                            single_packet=False,
                            queue_num=g,
                        )
                    m_sbs = []
                    for g in range(NG):
                        mb = int(m_base[g, s])
                        ncol = int(n_mm[g, s]) * P
                        m_sb = mpool.tile([P, ncol], bf16, name=f"m{g}", tag=f"m{g}")
                        nc.sync.dma_start(
                            out=m_sb[:], in_=m_d[:, mb : mb + ncol]
                        )
                        m_sbs.append(m_sb)
                    ps_tiles = {}
                    for g in range(NG):
                        par = g % 2
                        for C, entries in enumerate(sched[(g, s)]):
                            for i, o in entries:
                                w = GW * s + i
                                if first_gc[w] == (g, C):
                                    ps_tiles[w] = ppool.tile(
                                        [P, d], f32, space="PSUM", name="ps"
                                    )
                                is_stop = last_gc[w] == (g, C)
                                nc.tensor.matmul(
                                    out=ps_tiles[w][:],
                                    lhsT=m_sbs[g][:, o * P : (o + 1) * P],
                                    rhs=gb3[g][:, C, par * d : (par + 1) * d],
                                    start=(first_gc[w] == (g, C)),
                                    stop=is_stop,
                                )
                                if is_stop:
                                    nc.scalar.copy(
                                        out=out_sb[:, w * d : (w + 1) * d],
                                        in_=ps_tiles[w][:],
                                    )
                    nc.sync.dma_start(
                        out=out_d[:, s * GW * d : (s + 1) * GW * d],
                        in_=out_sb[:, s * GW * d : (s + 1) * GW * d],
                    )

            if reps == 1:
                body()
            else:
                with tc.For_i(0, reps, 1):
                    body()
    nc.compile()
    return nc


def prep_shards(rows, cols, vals):
    rows = np.asarray(rows).astype(np.int64)
    cols = np.asarray(cols).astype(np.int64)
    vals = np.asarray(vals).astype(np.float32)
    e = rows.shape[0]

    k = rows // RPC
    lr = rows - k * RPC
    wv = lr // P
    rloc_v = lr - wv * P
    h = cols // HROWS
    par = cols % 2
    g = 2 * h + par
    idxloc = ((cols - h * HROWS) // 2).astype(np.int16)

    perm = np.lexsort((idxloc, wv, g, k))
    k_s, g_s, w_s = k[perm], g[perm], wv[perm]
    key = (k_s * NG + g_s) * W + w_s
    counts = np.bincount(key, minlength=NC * NG * W)
    u = counts.reshape(NC, NG, W).max(axis=0)  # [NG, W]

    sc = _schedule(u)
    off_gw, T_call, sched = sc["off_gw"], sc["T_call"], sc["sched"]
    n_mm, slot_base, m_base = sc["n_mm"], sc["slot_base"], sc["m_base"]
    n_slots, n_mcols = sc["n_slots"], sc["n_mcols"]

    # ordinal lookup: ord_flat[((g*NS + s)*64 + C)*8 + i] = ordinal or -1
    ord_flat = np.full(NG * NS * 64 * GW, -1, np.int64)
    for (g, s), percall in sched.items():
        for C, entries in enumerate(percall):
            for i, o in entries:
                ord_flat[((g * NS + s) * 64 + C) * GW + i] = o

    starts = np.concatenate([[0], np.cumsum(counts)])
    q = np.arange(e) - np.repeat(starts[:-1], counts)  # rank within (k,g,w) run
    s_e = w_s // GW
    i_e = w_s % GW
    call_slot = off_gw[g_s, w_s] + q
    C_e = call_slot // P
    p_e = call_slot % P
    slot = slot_base[g_s, s_e] + call_slot

    idx16 = np.zeros((NC, 16, n_slots // 16), np.int16)
    idx16[k_s, slot % 16, slot // 16] = idxloc[perm]

    # window pads (cnt..u) -> last valid idx of the run; call-tail pads
    # (S..T*128) -> last valid idx of the call's last window run.
    nrun = NC * NG * W
    run_k = np.arange(nrun) // (NG * W)
    run_g = (np.arange(nrun) // W) % NG
    run_w = np.arange(nrun) % W
    run_s = run_w // GW
    has = counts > 0
    last_idx = np.zeros(nrun, np.int16)
    last_idx[has] = idxloc[perm][starts[1:][has] - 1]
    run_u = u[run_g, run_w]
    pad_n = run_u - counts
    run_base = slot_base[run_g, run_s] + off_gw[run_g, run_w]
    pad_slot = (
        np.repeat(run_base + counts, pad_n)
        + np.concatenate([np.arange(n) for n in pad_n])
    )
    pad_k = np.repeat(run_k, pad_n)
    idx16[pad_k, pad_slot % 16, pad_slot // 16] = np.repeat(last_idx, pad_n)
    # call tails
    tail_slots = []
    tail_ks = []
    tail_vals = []
    for g in range(NG):
        for s in range(NS):
            S = int(sc["S_call"][g, s])
            Tp = int(T_call[g, s]) * P
            if Tp > S:
                lastw = GW * s + GW - 1
                for kk in range(NC):
                    r = ((kk * NG + g) * W) + lastw
                    tail_slots.append(
                        np.arange(slot_base[g, s] + S, slot_base[g, s] + Tp)
                    )
                    tail_ks.append(np.full(Tp - S, kk))
                    tail_vals.append(np.full(Tp - S, last_idx[r], np.int16))
    if tail_slots:
        ts_ = np.concatenate(tail_slots)
        tk_ = np.concatenate(tail_ks)
        tv_ = np.concatenate(tail_vals)
        idx16[tk_, ts_ % 16, ts_ // 16] = tv_

    idx128 = np.tile(idx16, (1, 8, 1))

    # M host
    o_e = ord_flat[((g_s * NS + s_e) * 64 + C_e) * GW + i_e]
    assert (o_e >= 0).all()
    mcol = m_base[g_s, s_e] + o_e * P + rloc_v[perm]
    m_host = np.zeros((NC, P, n_mcols), BF16)
    m_host[k_s, p_e, mcol] = vals[perm].astype(BF16)
    return idx128, m_host, u


def make_in_maps(rows, cols, vals, embeds):
    idx128, m_host, u = prep_shards(rows, cols, vals)
    emb = np.asarray(embeds).astype(np.float32)
    tabs = [
        np.ascontiguousarray(
            emb[h * HROWS : (h + 1) * HROWS].astype(BF16).reshape(NPAIR, 2 * D)
        )
        for h in range(2)
    ]
    in_maps = []
    for c in range(NC):
        m = {f"tab{h}": tabs[h] for h in range(2)}
        m["idx"] = np.ascontiguousarray(idx128[c])
        m["m"] = np.ascontiguousarray(m_host[c])
        in_maps.append(m)
    return in_maps, u


def kernel(rows, cols, vals, embeds):
    global LAST_RESULTS
    from concourse.bass_utils import run_bass_kernel_spmd

    in_maps, u = make_in_maps(rows, cols, vals, embeds)
    nc = build_program(u)

    res = run_bass_kernel_spmd(
        nc,
        in_maps,
        core_ids=list(range(NC)),
        trace=bool(int(os.environ.get("GCN_TRACE", "0"))),
    )
    LAST_RESULTS = res

    blocks = []
    for c in range(NC):
        o = res.results[c]["out"].reshape(P, W, D)
        blocks.append(o.transpose(1, 0, 2).reshape(W * P, D)[:RPC])
    return np.ascontiguousarray(np.concatenate(blocks, axis=0), dtype=np.float32)

